# revision 44
# baseline (speedup 1.0000x reference)
"""Trainium2 Bass kernel for nn_GAT_87617332838818.

Mathematical collapse: softmax weights sum to 1 per destination segment and
the aggregated message hp[dst] is constant within the segment, so the GAT
message passing is the identity and the network reduces to a per-node MLP:

    logits = W2r @ elu(W1r @ elu(W0r @ x^T))

Device strategy (8 NeuronCores, node-sharded 6250 rows each), v2:
  - exact branch-free ELU:  elu(p) = max(p, 0) + min(exp(p) - 1, 0)
    so each hidden layer needs ONE matmul (on elu directly, no bias or
    +1-inflation anywhere) plus only TWO elementwise passes:
      ACT:  e = exp(p)                       (PSUM -> SBUF fp16, ~540 ns)
      DVE:  h = relu(p) + min(e - 1, 0)      (custom fused DVE op
                                              ELU_COMBINE_ANT, ~560 ns)
    The custom op is authored + registered at import (4 ALU stages).
  - output pairs packed rows 0:40 / 64:104 of one PSUM bank (base 0/64),
    single Identity drain per pair on ACT, fp16 DMA out.
  - every x-batch DMA issued up front (batch 0 at top priority, carries
    W0); 5-stage pipeline skew so no stage consumes same-tick data;
    steady state is ACT-bound at ~1.35 us per 512-node group.
  - tapering tail groups (320/192/106) shorten the post-matmul drain.
  - NOTE: the tile scheduler floats dependency-free instructions early;
    mid-stream junk matmuls (PE p-state fillers) backfired and stay off.
  - all data fp16 (x, weights, activations); f32 only inside PSUM.
"""

import os
import sys

import numpy as np

for _p in ("/root/.axon_site/_ro/trn_rl_repo", "/opt/trn_rl_repo"):
    if os.path.isdir(_p) and _p not in sys.path:
        sys.path.append(_p)

import concourse.bass as bass
import concourse.tile as tile
from concourse import bacc, mybir
from concourse import dve_ops as _dve_ops
from concourse.bass_utils import run_bass_kernel_spmd
from concourse.dve_spec import (
    C0,
    Spec,
    Src0,
    Src1,
    Zero,
    _has_src1,
    lower as _dve_lower,
    minn,
    relu as _dve_relu_sym,
)
from concourse.dve_uop import DveOpSpec as _DveOpSpec


def _register_elu_combine():
    """Author a fused custom DVE op: out = relu(in0) + min(in1 - s0, 0).

    Computes the branch-free ELU  h = max(p,0) + min(e-1,0)  in ONE Vector
    pass (in0 = pre-activation PSUM, in1 = exp(p) from ACT), replacing a
    tensor_scalar + scalar_tensor_tensor pair.
    """
    name = "ELU_COMBINE_ANT"
    for op in _dve_ops.OPS:
        if op.name == name:
            return op
    spec = Spec(
        body=_dve_relu_sym(Src0) + minn(Src1 - C0, Zero),
        reference=lambda in0, in1, s0, s1, imm2: np.maximum(
            in0.astype(np.float32), 0
        ) + np.minimum(in1.astype(np.float32) - s0, 0),
    )
    shas = {}
    for ver in ("v3", "v4"):
        tmp = _DveOpSpec(name=name, opcode=0, uops=_dve_lower(spec, ver=ver),
                         rd1_en=_has_src1(spec))
        shas[ver] = tmp.sha(ver)
    op = _dve_ops.DveOp(name=name, spec=spec, subdim=False, uops_sha=shas)
    _dve_ops.OPS.append(op)
    _dve_ops.CUSTOM_DVE_SPECS[name] = spec
    _dve_ops._SUB_OPCODE_FOR_NAME[name] = (
        max(_dve_ops._SUB_OPCODE_FOR_NAME.values()) + 1
    )
    assert _dve_ops._SUB_OPCODE_FOR_NAME[name] < 0x20
    return op


_ELU_OP = _register_elu_combine()

N_CORES = 8
N_PER = 6250            # 50000 / 8
D_IN = 128
D_HID = 96
D_OUT = 40
FDP = 512               # group free-dim (1 PSUM bank)

F16 = mybir.dt.float16
F32 = mybir.dt.float32

Act = mybir.ActivationFunctionType
Alu = mybir.AluOpType

# 11 full groups + a tapering tail: the last groups are small so the
# pipeline-drain chain after the final matmul is short.
_pairs = [FDP] * 11 + [320, 192, 106]
assert sum(_pairs) == N_PER
P = len(_pairs)
_pstarts = [sum(_pairs[:i]) for i in range(P)]
NPAIR = (P + 1) // 2

# ---- schedule knobs ----
# (group, layer) where relu runs on ACT and the add on DVE (fp16 2x)
Y_INST = ()
# output pairs drained on ACT instead of DVE
DRAIN_ON_ACT = tuple(k for k in range(NPAIR) if k % 2 == 0)
N_WARM = 4              # junk matmuls before the real stream (PE ramp)
JUNK0, JUNK1, JUNK2 = 0, 0, 0   # junk matmuls after mm0/mm1/mm2
                                # (the tile scheduler floats dep-free junk
                                # ahead of real matmuls - keep these 0)
X_BATCHES = [1, 1, 2, 4, 6]
H_BUFS = 4

_batch_of = {}
_b0 = 0
for _bi, _bn in enumerate(X_BATCHES):
    for _g in range(_b0, min(_b0 + _bn, P)):
        _batch_of[_g] = _bi
    _b0 += _bn
assert _b0 >= P


def _build_program() -> bass.Bass:
    nc = bacc.Bacc(None, target_bir_lowering=False, debug=False)

    # xw packs [w0t | xT]: cols 0..95 = W0^T fp16, cols 96.. = x^T shard
    xw = nc.declare_dram_parameter("xw", [D_IN, D_HID + N_PER], F16,
                                   isOutput=False)
    # wb packs [w1t | w2t] fp16
    wb = nc.declare_dram_parameter("wb", [D_HID, D_HID + D_OUT], F16,
                                   isOutput=False)
    # packed output: pair k at cols [512k, 512k+512): rows 0:40 = group 2k,
    # rows 64:104 = group 2k+1 (rows 40:64 unused). Host unpacks.
    yT = nc.declare_dram_parameter("yT", [104, FDP * NPAIR], F16,
                                   isOutput=True)

    st = {}
    st_batch = {}
    batch_tiles = {}

    with tile.TileContext(nc) as tc:
        with (
            tc.tile_pool(name="consts", bufs=1) as consts,
            tc.tile_pool(name="x0", bufs=1) as x0pool,
            tc.tile_pool(name="xin", bufs=2) as xpool,
            tc.tile_pool(name="e", bufs=3) as epool,
            tc.tile_pool(name="h", bufs=H_BUFS) as hpool,
            tc.tile_pool(name="o", bufs=4) as opool,
            tc.tile_pool(name="ps0", bufs=3, space="PSUM") as ps0,
            tc.tile_pool(name="ps1", bufs=2, space="PSUM") as ps1,
            tc.tile_pool(name="ps2", bufs=2, space="PSUM") as ps2,
            tc.tile_pool(name="psj", bufs=1, space="PSUM") as psj,
        ):
            junk_in = consts.tile([D_IN, FDP], F16, tag="junkin")
            nc.vector.memset(junk_in[:], 0.0)
            junkp = psj.tile([D_HID, FDP], F32, tag="junkp")

            wb_sb = consts.tile([D_HID, D_HID + D_OUT], F16, tag="wb")
            w1_sb = wb_sb[:, :D_HID]
            w2_sb = wb_sb[:, D_HID:D_HID + D_OUT]

            # PE ramp warm-up on junk data during the DMA-bound head
            for _ in range(N_WARM):
                nc.tensor.matmul(junkp[:], junk_in[:, :D_HID], junk_in[:],
                                 start=True, stop=True)

            def junk_mm(w_ap, n):
                k, m = w_ap.shape
                for _ in range(n):
                    nc.tensor.matmul(junkp[:m, :], w_ap,
                                     junk_in[:k, :], start=True, stop=True)

            def load_batch(bi):
                """Issue one x-batch DMA (Sync queue).  Batch 0 carries w0
                and is pinned to the highest priority so its transfer is
                never delayed behind the bigger batches."""
                g0 = sum(X_BATCHES[:bi])
                if g0 >= P:
                    return
                g1 = min(g0 + X_BATCHES[bi], P)
                lo = _pstarts[g0] + (0 if bi else -D_HID)  # b0 incl. w0
                hi = _pstarts[g1 - 1] + _pairs[g1 - 1]
                xt = x0pool.tile([D_IN, hi - lo], F16, tag=f"xt{bi}")
                if bi == 0:
                    with tc.high_priority():
                        nc.sync.dma_start(xt[:],
                                          xw[:, D_HID + lo:D_HID + hi])
                else:
                    nc.sync.dma_start(xt[:], xw[:, D_HID + lo:D_HID + hi])
                st_batch[bi] = {"xt": xt, "base": lo}

            def stage_load(p):
                st[p] = dict(st_batch[_batch_of[p]])

            def stage_mm0(p):
                fd = _pairs[p]
                s = st[p]
                xo = _pstarts[p] - s["base"]
                p0 = ps0.tile([D_HID, FDP], F32, tag="p0")
                nc.tensor.matmul(p0[:, :fd], batch_tiles["w0"],
                                 s["xt"][:, xo:xo + fd],
                                 start=True, stop=True)
                junk_mm(batch_tiles["w0"], JUNK0)
                s["p0"] = p0

            def stage_elu(p, lyr):
                """h = max(p,0) + min(exp(p)-1, 0) from PSUM, fp16 out."""
                fd = _pairs[p]
                s = st[p]
                psum = s.pop(f"p{lyr}")
                e = epool.tile([D_HID, FDP], F16, tag=f"e{lyr}")
                h = hpool.tile([D_HID, FDP], F16, tag=f"h{lyr + 1}")
                nc.scalar.activation(e[:, :fd], psum[:, :fd], Act.Exp)
                nc.vector._custom_dve(_ELU_OP, out=h[:, :fd],
                                      in0=psum[:, :fd], in1=e[:, :fd],
                                      s0=1.0)
                s[f"h{lyr + 1}"] = h

            def stage_mm1(p):
                fd = _pairs[p]
                s = st[p]
                p1 = ps1.tile([D_HID, FDP], F32, tag="p1")
                nc.tensor.matmul(p1[:, :fd], w1_sb, s.pop("h1")[:, :fd],
                                 start=True, stop=True)
                junk_mm(w1_sb, JUNK1)
                s["p1"] = p1

            pair_state = {}

            def stage_mm2(p):
                fd = _pairs[p]
                s = st.pop(p)
                if p % 2 == 0:
                    p2 = ps2.tile([104, FDP], F32, tag="p2")
                    pair_state[p // 2] = p2
                    rows = slice(0, D_OUT)
                else:
                    p2 = pair_state[p // 2]
                    rows = slice(64, 64 + D_OUT)
                nc.tensor.matmul(p2[rows, :fd], w2_sb, s.pop("h2")[:, :fd],
                                 start=True, stop=True)
                junk_mm(w2_sb, JUNK2)

            def stage_out(p):
                """Drain + DMA a completed output pair (runs one tick after
                the pair's last matmul so it never head-of-line blocks the
                next tick's elementwise work)."""
                if not ((p % 2 == 1) or (p == P - 1)):
                    return
                nrows = 104 if p % 2 == 1 else D_OUT
                kp = p // 2
                p2 = pair_state.pop(kp)
                ow = max(_pairs[2 * kp:2 * kp + 2])
                o = opool.tile([104, FDP], F16, tag="o")
                if kp in DRAIN_ON_ACT:
                    nc.scalar.activation(o[:nrows, :ow], p2[:nrows, :ow],
                                         Act.Identity)
                else:
                    nc.vector.tensor_copy(o[:nrows, :ow], p2[:nrows, :ow])
                nc.sync.dma_start(yT[:, kp * FDP:kp * FDP + ow], o[:, :ow])

            # deep skew: every stage consumes data produced a full tick
            # earlier, so no same-tick cross-engine chains throttle the
            # cadence down to the critical path.
            for pp in range(P + 6):
                if pp == 0:
                    load_batch(0)
                    load_batch(1)
                    batch_tiles["w0"] = st_batch[0]["xt"][:, 0:D_HID]
                    nc.sync.dma_start(wb_sb[:], wb[:])
                elif pp <= len(X_BATCHES) - 2:
                    load_batch(pp + 1)
                if pp < P:
                    stage_load(pp)
                if 0 <= pp - 1 < P:
                    stage_mm0(pp - 1)
                if 0 <= pp - 2 < P:
                    stage_elu(pp - 2, 0)
                if 0 <= pp - 3 < P:
                    stage_mm1(pp - 3)
                if 0 <= pp - 4 < P:
                    stage_elu(pp - 4, 1)
                if 0 <= pp - 5 < P:
                    stage_mm2(pp - 5)
                    if pp - 5 == P - 1:
                        # last pair: nothing left to head-of-line block,
                        # drain immediately instead of next tick
                        stage_out(P - 1)
                if 0 <= pp - 6 < P - 1:
                    stage_out(pp - 6)

    nc.compile()
    return nc


_prog_cache = []
last_result = None


def kernel(**inputs) -> np.ndarray:
    global last_result
    x = np.asarray(inputs["x"], np.float32)           # [50000, 128]
    W0 = np.asarray(inputs["W0"], np.float32).reshape(D_HID, D_IN)
    W1 = np.asarray(inputs["W1"], np.float32).reshape(D_HID, D_HID)
    W2 = np.asarray(inputs["W2"], np.float32).reshape(D_OUT, D_HID)

    n = x.shape[0]
    assert n == N_CORES * N_PER, f"unexpected node count {n}"

    xT16 = x.T.astype(np.float16)                            # [128, 50000]
    w0t = W0.T.astype(np.float16)                            # [128, 96]
    wbm = np.ascontiguousarray(
        np.concatenate([W1.T, W2.T], axis=1).astype(np.float16))  # [96,136]

    if not _prog_cache:
        _prog_cache.append(_build_program())
    nc = _prog_cache[0]

    in_maps = []
    for i in range(N_CORES):
        xwi = np.ascontiguousarray(
            np.concatenate([w0t, xT16[:, i * N_PER:(i + 1) * N_PER]], axis=1))
        in_maps.append(dict(xw=xwi, wb=wbm))
    res = run_bass_kernel_spmd(nc, in_maps, list(range(N_CORES)))
    last_result = res
    out = np.empty((n, D_OUT), np.float32)
    for i in range(N_CORES):
        yt = np.asarray(res.results[i]["yT"], np.float32)  # [104, 512*NPAIR]
        base = i * N_PER
        for kp in range(NPAIR):
            c0 = kp * FDP
            g0 = 2 * kp
            w0_ = _pairs[g0]
            out[base + _pstarts[g0]:base + _pstarts[g0] + w0_] = \
                yt[0:D_OUT, c0:c0 + w0_].T
            if g0 + 1 < P:
                w1_ = _pairs[g0 + 1]
                out[base + _pstarts[g0 + 1]:base + _pstarts[g0 + 1] + w1_] = \
                    yt[64:64 + D_OUT, c0:c0 + w1_].T
    return out


if __name__ == "__main__":
    data = np.load("/tmp/gat_inputs.npz")
    y = kernel(**{k: data[k] for k in data.files})
    print("out", y.shape, y.dtype, "absmax", np.abs(y).max())
